# revision 41
# baseline (speedup 1.0000x reference)
"""Trainium2 Bass kernel for nn_CoreferenceResolver (coref UNet + pair decoder).

Sharding: core c handles batch b=c//2 and pair-half h=c%2 (496 of 992 pairs).
The gather/cosine/UNet stages are replicated on the two cores sharing a batch;
the extractor linears and group-bilinear decoder are sharded over pairs.

Weights are packed host-side into bf16 blobs + two streamed bf16 W1 tensors so
the whole weight set moves in ~20 DMAs. The cosine matrix is computed as a
masked gram matrix (entity norms come off its diagonal), enc1 runs on a
3-partition column im2col, dec1 uses a 4-phase decomposition of the upsample
conv, and the attention gates apply their 1x1 convs before upsampling.
"""
import os
import sys

for _p in ("/opt/trn_rl_repo",):
    if os.path.isdir(_p) and _p not in sys.path:
        sys.path.insert(0, _p)

import numpy as np

import concourse.bass as bass
import concourse.tile as tile
from concourse import bacc, mybir
from concourse.bass_utils import run_bass_kernel_spmd

f32 = mybir.dt.float32
i16 = mybir.dt.int16
AF = mybir.ActivationFunctionType
OP = mybir.AluOpType
f32r = mybir.dt.float32r
bf16 = mybir.dt.bfloat16


def _r(ap):
    """View an fp32 AP as float32r for full-rate PE streaming."""
    return ap.bitcast(f32r)

B, L, D, H = 4, 1024, 768, 12
NE, P = 32, 992
BLOCK = 64
G = D // BLOCK          # 12 groups
OUT_CH = 256
NCORES = 8
NH = P // 2             # 496 pairs per core
KD = D // 128           # 6 chunks of the D dim


def _mk_layout(entries):
    cols = {}
    c = 0
    for name, n in entries:
        cols[name] = (c, n)
        c += n
    return cols, c

BA_COLS, CA = _mk_layout([
    ("enc1", 3 * 64),       # [3(dx), 3(dy)*64]
    ("enc2", 9 * 128),      # [64, 9, 128]
])

BA2_COLS, CA2 = _mk_layout([
    ("bott", 9 * 256),      # [128, 9, 256]
    ("ag2wg", 2 * 128),     # [128, 2, 128]
    ("ag2wx", 128),         # [128, 128]
    ("ag2psi", 1),          # [128, 1]
    ("smat", 2),            # [128, 2]
])

BB_COLS, CB = _mk_layout([
    ("dec2", 3 * 9 * 128),  # [128, 3, 9, 128]
])

BB2_COLS, CB2 = _mk_layout([
    ("ag1wg", 64),          # [128, 64]
    ("ag1wx", 64),          # [64, 64]
    ("ag1psi", 1),          # [64, 1]
    ("d1ph", 4 * 4 * 64),   # [128, 4(phase), 4(cell), 64]
    ("d1att", 9 * 64),      # [64, 9, 64]
    ("fin", 256),           # [64, 256]
])

BC_COLS, CC = _mk_layout([
    ("w2h", 2 * 768),       # [128, 2, 768]
])

BC2_COLS, CC2 = _mk_layout([
    ("w2t", 2 * 768),       # [128, 2, 768]
    ("wdec", G * 128),      # [128, G, 128]
])

FA_COLS, CFA = _mk_layout([
    ("ident", NE),          # [32, 32]
    ("mask", 1),            # [32, 1]
])

F32_COLS, CF = _mk_layout([
    ("enc1b", 1),           # [64, 1]
    ("enc2b", 1),           # [128, 1]
    ("bottb", 2),           # [128, 2]
    ("dec2b", 1),           # [128, 1]
    ("dec1b", 1),           # [64, 1]
    ("finb", 2),            # [128, 2]
    ("hbp", KD),            # [128, KD]
    ("tbp", KD),            # [128, KD]
    ("decb", 1),            # [2, 1]
    ("iota", 1),            # [32, 1]
    ("hi_f", NH),           # [1, NH]
    ("ti_f", NH),           # [1, NH]
])

I16_COLS, CI = _mk_layout([
    ("pidx", NH // 16),     # amap pair gather
])


def build_nc():
    nc = bacc.Bacc("TRN2", target_bir_lowering=False, debug=False, num_devices=NCORES)

    def inp(name, shape, dt=f32):
        return nc.dram_tensor(name, shape, dt, kind="ExternalInput")

    ent_in = inp("ent_in", [NE, D], f32r)
    i16b  = inp("i16b", [128, CI], i16)
    f32a  = inp("f32a", [32, CFA])
    f32b  = inp("f32b", [128, CF])
    blobA = inp("blobA", [128, CA], bf16)
    blobA2 = inp("blobA2", [128, CA2], bf16)
    blobB = inp("blobB", [128, CB], bf16)
    blobB2 = inp("blobB2", [128, CB2], bf16)
    blobC = inp("blobC", [128, CC], bf16)
    blobC2 = inp("blobC2", [128, CC2], bf16)
    W1h   = inp("W1h", [128, KD, D], bf16)
    W1t   = inp("W1t", [128, KD, D], bf16)
    # loaded as two [128, 3, D] pieces each to cut HWDGE issue slots

    y = nc.dram_tensor("y", [2, NH], f32, kind="ExternalOutput")

    from contextlib import ExitStack
    with tile.TileContext(nc) as tc, ExitStack() as _ctx:
        sbw = _ctx.enter_context(tc.tile_pool(name="sbw", bufs=1))   # persistent
        sbt = _ctx.enter_context(tc.tile_pool(name="sbt", bufs=4))   # rotating temps

        # ------------- DMA issue: SP queue in schedule order ---------------
        t_ent = sbw.tile([NE, D], f32r, tag="ent")
        nc.sync.dma_start(t_ent[:], ent_in[:])
        t_fa = sbw.tile([32, CFA], f32, tag="fa")
        nc.sync.dma_start(t_fa[:], f32a[:])
        t_bA = sbw.tile([128, CA], bf16, tag="bA")
        nc.sync.dma_start(t_bA[:], blobA[:])
        t_f32 = sbw.tile([128, CF], f32, tag="f32")
        nc.sync.dma_start(t_f32[:], f32b[:])
        t_i16 = sbw.tile([128, CI], i16, tag="i16")
        nc.sync.dma_start(t_i16[:], i16b[:])
        s_cos = sbw.tile([NE, NE], bf16, tag="scos")
        IC3   = sbw.tile([3, 34, 34], bf16, tag="IC3")
        w1h_s = []
        for k in range(KD):
            t = sbw.tile([128, D], bf16, tag=f"w1h{k}")
            nc.sync.dma_start(t[:], W1h[:, k, :])
            w1h_s.append(t)
        t_bA2 = sbw.tile([128, CA2], bf16, tag="bA2")
        nc.sync.dma_start(t_bA2[:], blobA2[:])
        w1t_s = []
        for k in range(KD):
            t = sbw.tile([128, D], bf16, tag=f"w1t{k}")
            nc.sync.dma_start(t[:], W1t[:, k, :])
            w1t_s.append(t)
        t_bB = sbw.tile([128, CB], bf16, tag="bB")
        nc.sync.dma_start(t_bB[:], blobB[:])
        t_bB2 = sbw.tile([128, CB2], bf16, tag="bB2")
        nc.sync.dma_start(t_bB2[:], blobB2[:])
        t_bC = sbw.tile([128, CC], bf16, tag="bC")
        nc.sync.dma_start(t_bC[:], blobC[:])
        t_bC2 = sbw.tile([128, CC2], bf16, tag="bC2")
        nc.sync.dma_start(t_bC2[:], blobC2[:])

        def bA(name, parts=128):
            if name in BA_COLS:
                c0, n = BA_COLS[name]
                return t_bA[0:parts, c0:c0 + n]
            c0, n = BA2_COLS[name]
            return t_bA2[0:parts, c0:c0 + n]

        def bB(name, parts=128):
            if name in BB_COLS:
                c0, n = BB_COLS[name]
                return t_bB[0:parts, c0:c0 + n]
            c0, n = BB2_COLS[name]
            return t_bB2[0:parts, c0:c0 + n]

        def bC(name, parts=128):
            if name in BC_COLS:
                c0, n = BC_COLS[name]
                return t_bC[0:parts, c0:c0 + n]
            c0, n = BC2_COLS[name]
            return t_bC2[0:parts, c0:c0 + n]

        def bF(name, parts=128):
            if name in FA_COLS:
                c0, n = FA_COLS[name]
                return t_fa[0:parts, c0:c0 + n]
            c0, n = F32_COLS[name]
            return t_f32[0:parts, c0:c0 + n]

        # ------------- persistent SBUF intermediates -----------------------
        entT  = sbw.tile([128, KD, NE], bf16, tag="entT")
        gram  = sbw.tile([NE, NE], f32, tag="gram")
        c1p   = sbw.tile([64, 32, 32], bf16, tag="c1p")    # dense
        p1p   = sbw.tile([64, 18, 18], bf16, tag="p1p")    # padded
        c2p   = sbw.tile([128, 16, 16], bf16, tag="c2p")   # dense
        p2p   = sbw.tile([128, 10, 10], bf16, tag="p2p")   # padded
        c3a   = sbw.tile([128, 8, 8], bf16, tag="c3a")     # dense
        c3b   = sbw.tile([128, 8, 8], bf16, tag="c3b")
        u2p0  = sbw.tile([128, 18, 18], bf16, tag="u2p0")  # padded
        u2p1  = sbw.tile([128, 18, 18], bf16, tag="u2p1")
        att2p = sbw.tile([128, 18, 18], bf16, tag="att2p")
        d2pad = sbw.tile([128, 18, 18], bf16, tag="d2pad")
        att1p = sbw.tile([64, 34, 34], bf16, tag="att1p")
        d1s   = sbw.tile([64, 32, 32], bf16, tag="d1s")    # dense
        amap0 = sbw.tile([128, 1024], f32, tag="amap0")
        amap1 = sbw.tile([128, 1024], f32, tag="amap1")
        ew1   = sbw.tile([NE, D], f32, tag="ew1")
        et1   = sbw.tile([NE, D], f32, tag="et1")
        ohhi  = sbw.tile([NE, NH], f32, tag="ohhi")
        ohti  = sbw.tile([NE, NH], f32, tag="ohti")
        htT0  = sbw.tile([128, NH], bf16, tag="htT0")
        htT1  = sbw.tile([128, NH], bf16, tag="htT1")
        hsT   = sbw.tile([128, KD, NH], bf16, tag="hsT")
        tsT   = sbw.tile([128, KD, NH], bf16, tag="tsT")

        # ------------- Pool queue: memsets, later broadcasts/gathers -------
        ones_t = sbw.tile([1, 128], bf16, tag="ones")
        nc.gpsimd.memset(ones_t[:], 1.0)
        for t in (IC3, p1p, p2p, u2p0, u2p1, att2p, d2pad, att1p):
            nc.gpsimd.memset(t[:], 0.0)

        identr = sbt.tile([NE, NE], f32, tag="identr")
        nc.vector.tensor_copy(_r(identr[:]), bF("ident", NE))

        pu_cm = tc.tile_pool(name="pu", bufs=3, space="PSUM")
        pu = pu_cm.__enter__()
        pw_cm = tc.tile_pool(name="pw", bufs=1, space="PSUM")
        pw = pw_cm.__enter__()

        # ------------- transposes (masked raw entities) --------------------
        p_tT = pu.tile([128, KD, NE], f32, tag="pu")
        for k in range(KD):
            nc.tensor.transpose(_r(p_tT[:, k, :]), t_ent[:, k * 128:(k + 1) * 128],
                                _r(identr[:]))
        nc.vector.tensor_copy(entT[:], p_tT[:])

        # ------------- gram + cosine ---------------------------------------
        p_g = pu.tile([NE, NE], f32, tag="pu")
        for k in range(KD):
            nc.tensor.matmul(p_g[:], entT[:, k, :], entT[:, k, :],
                             start=(k == 0), stop=(k == KD - 1))
        nc.vector.tensor_copy(_r(gram[:]), p_g[:])
        # norms off the diagonal: ss = sum(gram * I)
        dd = sbt.tile([NE, NE], f32, tag="dd")
        nc.vector.tensor_mul(dd[:], gram[:], bF("ident", NE))
        ss = sbt.tile([NE, 1], f32, tag="ss")
        nc.vector.reduce_sum(ss[:], dd[:], axis=mybir.AxisListType.X)
        nrmv = sbt.tile([NE, 1], f32, tag="nrmv")
        nc.scalar.sqrt(nrmv[:], ss[:])
        nc.vector.tensor_single_scalar(nrmv[:], nrmv[:], 1e-13, op=OP.max)
        rinv = sbt.tile([NE, 1], f32, tag="rinv")
        nc.vector.reciprocal(rinv[:], nrmv[:])
        nc.vector.tensor_tensor(out=rinv[:], in0=rinv[:], in1=bF("mask", NE),
                                op=OP.mult)
        diag_r = sbt.tile([NE, NE], f32, tag="diag_r")
        nc.vector.tensor_scalar(out=_r(diag_r[:]), in0=bF("ident", NE),
                                scalar1=rinv[:], scalar2=None, op0=OP.mult)
        # cos = D * (gram^T * D): transpose-with-diag then row scale
        p_ct = pu.tile([NE, NE], f32, tag="pu")
        nc.tensor.transpose(_r(p_ct[:]), _r(gram[:]), _r(diag_r[:]))
        nc.vector.tensor_scalar(out=s_cos[:], in0=p_ct[:],
                                scalar1=rinv[:], scalar2=None, op0=OP.mult)

        # IC3[dx, r, c] = cos[r-1, c+dx-2] (zero padded)
        for dx, q in ((0, nc.scalar), (1, nc.scalar), (2, nc.gpsimd)):
            c_lo = max(0, 2 - dx)
            c_hi = min(34, 34 - dx)
            ncol = c_hi - c_lo
            s_lo = c_lo + dx - 2
            q.dma_start(IC3[dx:dx + 1, 1:33, c_lo:c_lo + ncol],
                        s_cos[:, s_lo:s_lo + ncol])
        sgd = sbt.tile([NE, 1], f32, tag="sgd")
        nc.scalar.activation(sgd[:], ss[:], AF.Sigmoid)

        # ------------- premultiply ew = ent_masked @ W1 --------------------
        p_ew = pw.tile([NE, D], f32, tag="pw")

        def premult(ws, kc):
            for n0, n1 in ((0, 512), (512, 768)):
                nc.tensor.matmul(p_ew[:, n0:n1], entT[:, kc, :],
                                 ws[kc][:, n0:n1],
                                 start=(kc == 0), stop=(kc == KD - 1),
                                 skip_group_check=True)


        premult(w1h_s, 0)
        premult(w1h_s, 1)

        # ------------- enc1: 3 row-tap matmuls x 2 N-halves ----------------
        for hh in range(2):
            p_c1 = pu.tile([64, 512], f32, tag="pu")
            for dy in range(3):
                rr = slice(hh * 16 + dy, hh * 16 + dy + 16)
                nc.tensor.matmul(p_c1[:],
                                 bA("enc1", 3)[:, dy * 64:(dy + 1) * 64],
                                 IC3[:, rr, 1:33],
                                 start=(dy == 0), stop=(dy == 2))
            nc.scalar.activation(c1p[:, hh * 16:hh * 16 + 16, :],
                                 p_c1[:].rearrange("c (h w) -> c h w", h=16, w=32),
                                 AF.Relu, bias=bF("enc1b", 64))

        premult(w1h_s, 2)
        premult(w1h_s, 3)

        # ------------- pool1 -> p1p interior [64, 16, 16] ------------------
        tmp = sbt.tile([64, 16, 16], bf16, tag="t")
        nc.vector.tensor_max(tmp[:], c1p[:, 0:32:2, 0:32:2], c1p[:, 0:32:2, 1:32:2])
        nc.vector.tensor_max(tmp[:], tmp[:], c1p[:, 1:32:2, 0:32:2])
        nc.vector.tensor_max(p1p[:, 1:17, 1:17], tmp[:], c1p[:, 1:32:2, 1:32:2])

        # ------------- enc2: 9 shifted matmuls K=64 ------------------------
        p_c2 = pu.tile([128, 256], f32, tag="pu")
        e2w = bA("enc2", 64).rearrange("c (t m) -> c t m", t=9)
        for tap in range(9):
            dy, dx = tap // 3, tap % 3
            nc.tensor.matmul(p_c2[:], e2w[:, tap, :],
                             p1p[:, dy:dy + 16, dx:dx + 16],
                             start=(tap == 0), stop=(tap == 8))
        nc.scalar.activation(c2p[:],
                             p_c2[:].rearrange("c (h w) -> c h w", h=16, w=16),
                             AF.Relu, bias=bF("enc2b"))

        premult(w1h_s, 4)
        premult(w1h_s, 5)

        # one-hots for the pair gather of ew rows
        for (src_c, dst) in (("hi_f", ohhi), ("ti_f", ohti)):
            bc = sbt.tile([NE, NH], f32, tag="bc")
            nc.gpsimd.partition_broadcast(bc[:], bF(src_c, 1))
            nc.vector.tensor_scalar(out=_r(dst[:]), in0=bc[:],
                                    scalar1=bF("iota", NE), scalar2=None,
                                    op0=OP.is_equal)

        # ------------- pool2 -> p2p interior [128, 8, 8] -------------------
        tmp2 = sbt.tile([128, 8, 8], bf16, tag="t")
        nc.vector.tensor_max(tmp2[:], c2p[:, 0:16:2, 0:16:2], c2p[:, 0:16:2, 1:16:2])
        nc.vector.tensor_max(tmp2[:], tmp2[:], c2p[:, 1:16:2, 0:16:2])
        nc.vector.tensor_max(p2p[:, 1:9, 1:9], tmp2[:], c2p[:, 1:16:2, 1:16:2])

        # ------------- bottleneck: 9 taps x 2 M-chunks, K=128 --------------
        bw = bA("bott").rearrange("c (t m) -> c t m", t=9)
        for mc, dst in ((0, c3a), (1, c3b)):
            p_c3 = pu.tile([128, 64], f32, tag="pu")
            for tap in range(9):
                dy, dx = tap // 3, tap % 3
                nc.tensor.matmul(p_c3[:], bw[:, tap, mc * 128:(mc + 1) * 128],
                                 p2p[:, dy:dy + 8, dx:dx + 8],
                                 start=(tap == 0), stop=(tap == 8))
            nc.scalar.activation(dst[:], p_c3[:].rearrange("c (h w) -> c h w", h=8, w=8),
                                 AF.Relu, bias=bF("bottb")[:, mc:mc + 1])

        nc.vector.tensor_copy(_r(ew1[:]), p_ew[:])

        # ------------- up2 -> u2p interiors --------------------------------
        for src, dst in ((c3a, u2p0), (c3b, u2p1)):
            for i in range(2):
                for j in range(2):
                    nc.vector.tensor_copy(dst[:, 1 + i:17:2, 1 + j:17:2], src[:])

        # ------------- attention gate 2 (pre-upsample trick) ---------------
        wg2 = bA("ag2wg").rearrange("c (t m) -> c t m", t=2)
        p_q2 = pu.tile([128, 8, 8], f32, tag="pu")
        nc.tensor.matmul(p_q2[:], wg2[:, 0, :], c3a[:], start=True, stop=False)
        nc.tensor.matmul(p_q2[:], wg2[:, 1, :], c3b[:], start=False, stop=True)
        p_x2 = pu.tile([128, 16, 16], f32, tag="pu")
        nc.tensor.matmul(p_x2[:], bA("ag2wx"), c2p[:])
        q2s = sbt.tile([128, 8, 8], f32, tag="q2s")
        nc.scalar.activation(q2s[:], p_q2[:], AF.Copy)
        r2 = sbt.tile([128, 16, 16], bf16, tag="r2")
        for aa in range(2):
            for bb in range(2):
                nc.vector.tensor_tensor(out=r2[:, aa:16:2, bb:16:2],
                                        in0=p_x2[:, aa:16:2, bb:16:2],
                                        in1=q2s[:], op=OP.add)
        nc.vector.tensor_single_scalar(r2[:], r2[:], 0.0, op=OP.max)
        p_g2 = pu.tile([1, 256], f32, tag="pu")
        nc.tensor.matmul(p_g2[:], bA("ag2psi"), r2[:].rearrange("c h w -> c (h w)"))
        a2 = sbt.tile([1, 256], bf16, tag="a2")
        nc.scalar.activation(a2[:], p_g2[:], AF.Sigmoid)
        p_a2b = pu.tile([128, 256], f32, tag="pu")
        nc.tensor.matmul(p_a2b[:], ones_t[:], a2[:])
        nc.vector.tensor_mul(att2p[:, 1:17, 1:17],
                             p_a2b[:].rearrange("c (h w) -> c h w", h=16, w=16), c2p[:])

        # ------------- dec2: 9 taps x 3 K-chunks ---------------------------
        pd2_cm = tc.tile_pool(name="pd2", bufs=1, space="PSUM")
        pd2p = pd2_cm.__enter__()
        p_d2 = pd2p.tile([128, 256], f32, tag="pd2")
        d2w = bB("dec2").rearrange("c (s t m) -> c s t m", s=3, t=9)
        srcs2 = (u2p0, u2p1, att2p)
        n_mm = 0
        for kc in range(3):
            for tap in range(9):
                dy, dx = tap // 3, tap % 3
                nc.tensor.matmul(p_d2[:], d2w[:, kc, tap, :],
                                 srcs2[kc][:, dy:dy + 16, dx:dx + 16],
                                 start=(n_mm == 0), stop=(n_mm == 26),
                                 skip_group_check=True)
                n_mm += 1
        nc.scalar.activation(d2pad[:, 1:17, 1:17],
                             p_d2[:].rearrange("c (h w) -> c h w", h=16, w=16),
                             AF.Relu, bias=bF("dec2b"))
        pd2_cm.__exit__(None, None, None)

        # ------------- attention gate 1 (pre-upsample trick) ---------------
        p_q1 = pu.tile([64, 16, 16], f32, tag="pu")
        nc.tensor.matmul(p_q1[:], bB("ag1wg"), d2pad[:, 1:17, 1:17])
        q1s = sbt.tile([64, 16, 16], f32, tag="q1s")
        nc.scalar.activation(q1s[:], p_q1[:], AF.Copy)
        c1v = c1p[:].rearrange("c h w -> c (h w)")
        r1 = sbt.tile([64, 32, 32], bf16, tag="r1")
        r1v = r1[:].rearrange("c h w -> c (h w)")
        a1 = sbt.tile([1, 1024], bf16, tag="a1")
        r1ah = []
        for hh in range(2):
            p_x1 = pu.tile([64, 512], f32, tag="pu")
            nc.tensor.matmul(p_x1[:], bB("ag1wx", 64),
                             c1v[:, hh * 512:(hh + 1) * 512])
            r1a = sbt.tile([64, 8, 2, 16, 2], f32, tag=f"r1a{hh}")
            x1v = p_x1[:].rearrange("c (h a w b) -> c h a w b", h=8, a=2, w=16, b=2)
            for aa in range(2):
                for bb in range(2):
                    nc.vector.tensor_tensor(
                        out=r1a[:, :, aa, :, bb],
                        in0=x1v[:, :, aa, :, bb],
                        in1=q1s[:, 8 * hh:8 * hh + 8, :], op=OP.add)
            nc.scalar.activation(r1v[:, hh * 512:(hh + 1) * 512],
                                 r1a[:].rearrange("c h a w b -> c (h a w b)"),
                                 AF.Relu)
        pg1h = []
        for hh in range(2):
            p_g1 = pu.tile([1, 512], f32, tag="pu")
            nc.tensor.matmul(p_g1[:], bB("ag1psi", 64),
                             r1v[:, hh * 512:(hh + 1) * 512])
            pg1h.append(p_g1)
        for hh in range(2):
            nc.scalar.activation(a1[:, hh * 512:(hh + 1) * 512], pg1h[hh][:], AF.Sigmoid)
        pa1h = []
        for hh in range(2):
            p_a1b = pu.tile([64, 512], f32, tag="pu")
            nc.tensor.matmul(p_a1b[:], ones_t[:, 0:64],
                             a1[:, hh * 512:(hh + 1) * 512])
            pa1h.append(p_a1b)
        for hh in range(2):
            nc.vector.tensor_mul(
                att1p[:, 1 + 16 * hh:17 + 16 * hh, 1:33],
                pa1h[hh][:].rearrange("c (h w) -> c h w", h=16, w=32),
                c1p[:, 16 * hh:16 * hh + 16, :])

        # premult tail (W1t stream lands late UNet)
        premult(w1t_s, 0)
        premult(w1t_s, 1)

        # ------------- dec1: 4-phase (u-part 2x2 cells + att 9 taps) -------
        # u-cells go into a dedicated psum pool right after d2pad is ready so
        # they fill the PE while the gate-1 chain runs.
        d1ph = bB("d1ph").rearrange("c (p l m) -> c p l m", p=4, l=4)
        d1at = bB("d1att", 64).rearrange("c (t m) -> c t m", t=9)
        pd1_cm = tc.tile_pool(name="pd1", bufs=1, space="PSUM")
        pd1 = pd1_cm.__enter__()
        p_d1a = pd1.tile([64, 2, 16, 16], f32, tag="pd1a")
        p_d1b = pd1.tile([64, 2, 16, 16], f32, tag="pd1b")
        p_d1t = [p_d1a, p_d1b]
        for a in range(2):
            for b in range(2):
                ph_i = a * 2 + b
                p_d1 = p_d1t[a][:, b, :, :]
                n_mm = 0
                for cu in range(2):
                    for cv in range(2):
                        nc.tensor.matmul(p_d1, d1ph[:, ph_i, cu * 2 + cv, :],
                                         d2pad[:, cu + a:cu + a + 16,
                                               cv + b:cv + b + 16],
                                         start=(n_mm == 0), stop=False,
                                         skip_group_check=True)
                        n_mm += 1
        premult(w1t_s, 2)
        premult(w1t_s, 3)
        for a in range(2):
            for b in range(2):
                p_d1 = p_d1t[a][:, b, :, :]
                for tap in range(9):
                    dy, dx = tap // 3, tap % 3
                    nc.tensor.matmul(p_d1, d1at[:, tap, :],
                                     att1p[:, a + dy:a + dy + 31:2,
                                           b + dx:b + dx + 31:2],
                                     start=False, stop=(tap == 8),
                                     skip_group_check=True)
                nc.scalar.activation(d1s[:, a:32:2, b:32:2], p_d1,
                                     AF.Relu, bias=bF("dec1b", 64))
            if a == 0:
                premult(w1t_s, 4)
                premult(w1t_s, 5)

        nc.scalar.activation(_r(et1[:]), p_ew[:], AF.Copy)
        pd1_cm.__exit__(None, None, None)
        pw_cm.__exit__(None, None, None)

        # ------------- fin 1x1 conv -> amapT [256, 1024] -------------------
        d1v = d1s[:].rearrange("c h w -> c (h w)")
        c0, n = I16_COLS["pidx"]
        pidx = t_i16[:, c0:c0 + n]
        htT0x = sbt.tile([128, NH], f32, tag="htT0x")
        htT1x = sbt.tile([128, NH], f32, tag="htT1x")
        for mc, dst, htTx, htT in ((0, amap0, htT0x, htT0), (1, amap1, htT1x, htT1)):
            for hh in range(2):
                p_am = pu.tile([128, 512], f32, tag="pu")
                nc.tensor.matmul(p_am[:],
                                 bB("fin", 64)[:, mc * 128:(mc + 1) * 128],
                                 d1v[:, hh * 512:(hh + 1) * 512])
                if hh == 0:
                    nc.scalar.activation(dst[:, hh * 512:(hh + 1) * 512], p_am[:],
                                         AF.Identity, bias=bF("finb")[:, mc:mc + 1])
                else:
                    nc.vector.tensor_scalar(out=dst[:, hh * 512:(hh + 1) * 512],
                                            in0=p_am[:],
                                            scalar1=bF("finb")[:, mc:mc + 1],
                                            scalar2=None, op0=OP.add)
            nc.gpsimd.ap_gather(htTx[:].rearrange("c (n o) -> c n o", o=1),
                                dst[:].rearrange("c (n o) -> c n o", o=1), pidx,
                                channels=128, num_elems=1024, d=1, num_idxs=NH)
            nc.vector.tensor_copy(htT[:], htTx[:])

        pu_cm.__exit__(None, None, None)

        # ------------- pair features + decoder -----------------------------
        ph_cm = tc.tile_pool(name="ph", bufs=3, space="PSUM")
        ph = ph_cm.__enter__()
        pd_cm = tc.tile_pool(name="pd", bufs=4, space="PSUM")
        pd = pd_cm.__enter__()
        po_cm = tc.tile_pool(name="po", bufs=1, space="PSUM")
        po = po_cm.__enter__()
        p_out = po.tile([2, NH], f32, tag="po")
        w2h = bC("w2h").rearrange("c (t m) -> c t m", t=2)
        w2t = bC("w2t").rearrange("c (t m) -> c t m", t=2)
        wde = bC("wdec").rearrange("c (g m) -> c g m", g=G)
        for k in range(KD):
            cols = slice(k * 128, (k + 1) * 128)
            for (w2, ewt, oh, bp, dstT) in ((w2h, ew1, ohhi, "hbp", hsT),
                                            (w2t, et1, ohti, "tbp", tsT)):
                p_hs = ph.tile([128, NH], f32, tag="ph")
                nc.tensor.matmul(p_hs[:], _r(ewt[:, cols]), _r(oh[:]),
                                 start=True, stop=False)
                nc.tensor.matmul(p_hs[:], w2[:, 0, cols], htT0[:],
                                 start=False, stop=False)
                nc.tensor.matmul(p_hs[:], w2[:, 1, cols], htT1[:],
                                 start=False, stop=True)
                nc.scalar.activation(dstT[:, k, :], p_hs[:],
                                     AF.Tanh, bias=bF(bp)[:, k:k + 1])
            for half in range(2):
                g = 2 * k + half
                rows = slice(half * 64, (half + 1) * 64)
                p_u = pd.tile([128, NH], f32, tag="pd")
                nc.tensor.matmul(p_u[:], wde[rows, g, :], tsT[rows, k, :])
                v = sbt.tile([128, NH], bf16, tag="v")
                nc.vector.tensor_mul(v[0:64, :], p_u[0:64, :], hsT[rows, k, :])
                nc.vector.tensor_mul(v[64:128, :], p_u[64:128, :], hsT[rows, k, :])
                nc.tensor.matmul(p_out[:], bA("smat"), v[:],
                                 start=(g == 0), stop=(g == G - 1),
                                 skip_group_check=True)
        out_sb = sbt.tile([2, NH], f32, tag="out")
        nc.scalar.activation(out_sb[:], p_out[:], AF.Identity, bias=bF("decb", 2))
        nc.sync.dma_start(y[:], out_sb[:])
        po_cm.__exit__(None, None, None)
        pd_cm.__exit__(None, None, None)
        ph_cm.__exit__(None, None, None)

    nc.compile()
    return nc


def _wrap16(idx, n_slots):
    """int16 index layout for gpsimd gathers: wrapped in 16 partitions,
    replicated across the 8 gpsimd cores."""
    out = np.zeros((128, n_slots), np.int16)
    for j, v in enumerate(idx):
        out[np.arange(8) * 16 + j % 16, j // 16] = v
    return out


def f32r_round(a):
    """Round-to-nearest-even to fp32r (11 mantissa bits), matching the PE."""
    u = np.ascontiguousarray(a, np.float32).view(np.uint32).copy()
    u = (u + (np.uint32(0x7FF) + ((u >> np.uint32(12)) & np.uint32(1)))) & np.uint32(0xFFFFF000)
    return u.view(np.float32)


def pack_inputs(inputs):
    """Build the 8 per-core input maps from the full problem inputs."""
    import ml_dtypes
    x = np.asarray(inputs["x"], np.float32)
    entity_pos = np.asarray(inputs["entity_pos"])
    hts = np.asarray(inputs["hts"])

    def W(name):
        return np.asarray(inputs[name], np.float32)

    def blob(layout, ncols, parts_map):
        b = np.zeros((128, ncols), np.float32)
        for name, arr in parts_map.items():
            c0, n = layout[name]
            p = arr.shape[0]
            b[0:p, c0:c0 + n] = arr.reshape(p, n)
        return b

    shared = {}
    e1 = W("enc1_w").reshape(64, 9)            # [c, dy*3+dx]
    enc1 = np.zeros((3, 3 * 64), np.float32)   # [dx, dy*64+c]
    for dy in range(3):
        for dx in range(3):
            enc1[dx, dy * 64:(dy + 1) * 64] = e1[:, dy * 3 + dx]
    smat = np.zeros((128, 2), np.float32)
    smat[:64, 0] = 1.0
    smat[64:, 1] = 1.0
    blobA = blob(BA_COLS, CA, {
        "enc1": enc1,
        "enc2": W("enc2_w").reshape(128, 64, 9).transpose(1, 2, 0).copy(),
    })
    blobA2 = blob(BA2_COLS, CA2, {
        "bott": W("bott_w").reshape(256, 128, 9).transpose(1, 2, 0).copy(),
        "ag2wg": W("ag2_wg").reshape(128, 256).T.reshape(2, 128, 128).transpose(1, 0, 2).copy(),
        "ag2wx": W("ag2_wx").reshape(128, 128).T.copy(),
        "ag2psi": W("ag2_psi").reshape(1, 128).T.copy(),
        "smat": smat,
    })
    d1w = W("dec1_w")                          # [64, 192, 3, 3]
    du = d1w[:, 0:128]                         # u-part [64, 128, 3, 3]
    d1ph = np.zeros((128, 4, 4, 64), np.float32)
    taps_u = {(0, 0): [0], (0, 1): [1, 2], (1, 0): [0, 1], (1, 1): [2]}
    for a in range(2):
        for b_ in range(2):
            for cu in range(2):
                for cv in range(2):
                    acc = np.zeros((128, 64), np.float32)
                    for dy in taps_u[(a, cu)]:
                        for dx in taps_u[(b_, cv)]:
                            acc += du[:, :, dy, dx].T
                    d1ph[:, a * 2 + b_, cu * 2 + cv, :] = acc
    blobB = blob(BB_COLS, CB, {
        "dec2": W("dec2_w").reshape(128, 384, 9).transpose(1, 2, 0)
                .reshape(3, 128, 9, 128).transpose(1, 0, 2, 3).copy(),
    })
    blobB2 = blob(BB2_COLS, CB2, {
        "ag1wg": W("ag1_wg").reshape(64, 128).T.copy(),
        "ag1wx": W("ag1_wx").reshape(64, 64).T.copy(),
        "ag1psi": W("ag1_psi").reshape(1, 64).T.copy(),
        "d1ph": d1ph,
        "d1att": d1w[:, 128:192].reshape(64, 64, 9).transpose(1, 2, 0).copy(),
        "fin": W("fin_w").reshape(256, 64).T.copy(),
    })
    head_w = W("head_w"); tail_w = W("tail_w")
    wd = W("decoder_w").reshape(G, 64, 64, 2).transpose(2, 0, 3, 1).reshape(64, G, 128)
    blobC = blob(BC_COLS, CC, {
        "w2h": head_w[D:].reshape(2, 128, D).transpose(1, 0, 2).copy(),
    })
    blobC2 = blob(BC2_COLS, CC2, {
        "w2t": tail_w[D:].reshape(2, 128, D).transpose(1, 0, 2).copy(),
        "wdec": np.concatenate([wd, wd], axis=0).copy(),
    })
    for k, v in (("blobA", blobA), ("blobA2", blobA2), ("blobB", blobB),
                 ("blobB2", blobB2), ("blobC", blobC), ("blobC2", blobC2)):
        shared[k] = v.astype(ml_dtypes.bfloat16)
    shared["W1h"] = head_w[:D].reshape(KD, 128, D).transpose(1, 0, 2).astype(ml_dtypes.bfloat16)
    shared["W1t"] = tail_w[:D].reshape(KD, 128, D).transpose(1, 0, 2).astype(ml_dtypes.bfloat16)

    f32_shared = {
        "enc1b": W("enc1_b").reshape(64, 1),
        "enc2b": W("enc2_b").reshape(128, 1),
        "bottb": W("bott_b").reshape(2, 128).T.copy(),
        "dec2b": W("dec2_b").reshape(128, 1),
        "dec1b": W("dec1_b").reshape(64, 1),
        "finb": W("fin_b").reshape(2, 128).T.copy(),
        "hbp": W("head_b").reshape(KD, 128).T.copy(),
        "tbp": W("tail_b").reshape(KD, 128).T.copy(),
        "decb": W("decoder_b").reshape(2, 1),
        "iota": np.arange(NE, dtype=np.float32).reshape(NE, 1),
    }

    in_maps = []
    for c in range(NCORES):
        b, h = c // 2, c % 2
        m = dict(shared)
        start = entity_pos[b, :, 0].astype(np.int64)
        idx = np.minimum(start + 1, L - 1)
        mask = (start + 1 < L).astype(np.float32).reshape(NE, 1)
        m["ent_in"] = f32r_round(np.ascontiguousarray(x[b][idx]) * mask)
        f32al = np.zeros((32, CFA), np.float32)
        f32al[:, 0:NE] = np.eye(NE, dtype=np.float32)
        f32al[:, NE:NE + 1] = mask
        m["f32a"] = f32al
        fm = dict(f32_shared)
        f32bl = np.zeros((128, CF), np.float32)
        for name, arr in fm.items():
            c0, n = F32_COLS[name]
            p = arr.shape[0]
            f32bl[0:p, c0:c0 + n] = arr.reshape(p, n)
        hi = hts[b, h * NH:(h + 1) * NH, 0].astype(np.int64)
        ti = hts[b, h * NH:(h + 1) * NH, 1].astype(np.int64)
        f32bl[0, F32_COLS["hi_f"][0]:F32_COLS["hi_f"][0] + NH] = hi
        f32bl[0, F32_COLS["ti_f"][0]:F32_COLS["ti_f"][0] + NH] = ti
        m["f32b"] = f32bl
        i16bl = np.zeros((128, CI), np.int16)
        c0, n = I16_COLS["pidx"]
        i16bl[:, c0:c0 + n] = _wrap16((hi * NE + ti).astype(np.int16), NH // 16)
        m["i16b"] = i16bl
        in_maps.append(m)
    return in_maps


_NC_CACHE = None


def get_nc():
    global _NC_CACHE
    if _NC_CACHE is None:
        _NC_CACHE = build_nc()
    return _NC_CACHE


def kernel(**inputs):
    nc = get_nc()
    in_maps = pack_inputs(inputs)
    res = run_bass_kernel_spmd(nc, in_maps, core_ids=list(range(NCORES)))
    out = np.empty((B * P, 2), np.float32)
    for c in range(NCORES):
        b, h = c // 2, c % 2
        yc = res.results[c]["y"]                  # [2, NH]
        out[b * P + h * NH:b * P + (h + 1) * NH, :] = yc.T
    return out
